# revision 1
# baseline (speedup 1.0000x reference)
"""Two-layer GAT on 8 Trainium2 NeuronCores.

Strategy (edge/dst-partition parallel):
- Nodes are sharded 6250/core (core c owns nodes [c*6250,(c+1)*6250)). Within a
  core, its nodes are re-binned into 49 tiles of 128 "slots" with balanced total
  degree (host-side permutation; undone on host at the end).
- All edges (incl. self-loops) are assigned to the core/tile owning their dst.
- Per layer: each core projects its node slice (x @ W, plus folded attention
  columns), packs rows into a gather table, AllGathers the table, then per
  dst-tile: dma_gather of neighbor rows, segment softmax via one-hot matrices
  (T built on-device by is_equal-vs-iota; S uploaded from host), and the
  weighted aggregation as PSUM-accumulated matmuls.
- num_idxs for dma_gather are int16, so the table is gathered in two halves
  (rows < 25088 and >= 25088) with per-tile lo/hi edge groups.

Self-contained: only numpy/ml_dtypes/concourse imports; shapes hardcoded from
the problem spec; everything data-dependent is computed at runtime on host.
"""
import numpy as np
import ml_dtypes

import concourse.bass as bass
import concourse.bacc as bacc
import concourse.tile as tile
import concourse.mybir as mybir
from concourse import bass_utils

# problem constants
N = 50000
E = 500000
IN = 256
HID = 128
H1 = 2          # heads layer 1
OUT = 128
NEG = 0.2
NCORES = 8
NS = N // NCORES          # 6250 nodes per core
NT = 50                   # dst tiles per core (50*128 = 6400 slots)
SLOTS = NT * 128          # 6400
ROWS = NCORES * SLOTS     # 51200 global table rows
HALF = ROWS // 2          # 25600 (int16-safe gather bases)
EW1 = 384                 # table1 row: [1|h0|1|h1|a_s f32 x2] bf16 pad -> 768B
EW2 = 256                 # table2 row: [1|h2p|a_s2 f32] bf16 pad -> 512B
W = 128                   # one-hot window width (dst slots per chunk)

BF = ml_dtypes.bfloat16
DT = mybir.dt


# ----------------------------------------------------------------------------
# host preprocessing
# ----------------------------------------------------------------------------

# slot positions for the i-th biggest dst in a bin: spread big dsts out so the
# per-slot cumulative degree is near-linear (helps the window construction)
_SPREAD = np.argsort([(i % 16) * 8 + i // 16 for i in range(128)], kind="stable")
_SPREAD = np.array([(i % 16) * 8 + i // 16 for i in range(128)], np.int32)


def _balance_bins(deg_lo, deg_hi):
    """Assign NS dsts to NT bins of <=128 slots, balancing both the lo and hi
    edge counts (gather sections are padded separately per src-half).
    Returns (bin_of, slot_of)."""
    import heapq
    order = np.argsort(-(deg_lo + deg_hi), kind="stable")
    heap = [(0, 0, 0, t) for t in range(NT)]  # (max(lo,hi), lo, hi, t)
    heapq.heapify(heap)
    nfill = np.zeros(NT, np.int32)
    bin_of = np.zeros(NS, np.int32)
    slot_of = np.zeros(NS, np.int32)
    for d in order:
        popped = []
        while True:
            e = heapq.heappop(heap)
            if nfill[e[3]] < 128:
                break
            popped.append(e)
        for p in popped:
            heapq.heappush(heap, p)
        _, lo, hi, t = e
        bin_of[d] = t
        # i-th biggest dst of this bin lands on spread slot _SPREAD[i]
        slot_of[d] = _SPREAD[nfill[t]]
        nfill[t] += 1
        lo += int(deg_lo[d])
        hi += int(deg_hi[d])
        heapq.heappush(heap, (max(lo, hi), lo, hi, t))
    return bin_of, slot_of


def _w0_sched(cl):
    """Window start offsets for a section of cl chunks (PE base-partition
    constraint: starts must be multiples of 32 in {0, 32, 64})."""
    if W >= 128 or cl <= 1:
        return [0] * cl
    return [32 * round(c * 2 / (cl - 1)) for c in range(cl)]


def _fill_section(slots_sorted, cl):
    """Greedily pack edges (their dst slots, ascending) into cl chunks of 128
    with chunk c accepting only slots in [w0[c], w0[c]+W).
    Returns list of per-chunk edge-position lists, or None if infeasible."""
    w0 = _w0_sched(cl)
    n = len(slots_sorted)
    pos = 0
    chunks = []
    for c in range(cl):
        lim = w0[c] + W
        take = []
        while pos < n and len(take) < 128 and slots_sorted[pos] < lim:
            take.append(pos)
            pos += 1
        chunks.append(take)
        # feasibility: anything left behind must fit a later window
        if pos < n and c + 1 < cl and slots_sorted[pos] < w0[c + 1]:
            return None
    if pos < n:
        return None
    return chunks


def _wrap_idx(a):
    """[n] int16 -> [128, n//16], index i at partition i%16 col i//16, x8."""
    n = a.shape[0]
    return np.tile(a.reshape(n // 16, 16).T, (8, 1)).copy()


def _preprocess(edge_index):
    src = np.concatenate([edge_index[0], np.arange(N, dtype=np.int64)])
    dst = np.concatenate([edge_index[1], np.arange(N, dtype=np.int64)])
    core = (dst // NS).astype(np.int32)
    dl = (dst % NS).astype(np.int32)

    perm_rows = np.zeros(N, np.int64)   # node -> global table row
    binslot = np.zeros(N, np.int32)     # node -> local slot (bin*128+slot)
    src_is_lo = src < (N // 2)          # == (src core < 4) == (src_row < HALF)
    for c in range(NCORES):
        m = core == c
        deg_lo = np.bincount(dl[m & src_is_lo], minlength=NS)
        deg_hi = np.bincount(dl[m & ~src_is_lo], minlength=NS)
        b, s = _balance_bins(deg_lo, deg_hi)
        binslot[c * NS:(c + 1) * NS] = b * 128 + s
        perm_rows[c * NS:(c + 1) * NS] = c * SLOTS + b * 128 + s

    src_row = perm_rows[src]            # global row of src node
    dst_slot = binslot[dst]             # slot within owning (core,tile)
    tile_of = dst_slot // 128
    slot_in = dst_slot % 128
    lo = (src_row < HALF).astype(np.int32)

    # group edges by (core, tile, half); within a group sort by dst slot
    key = ((core.astype(np.int64) * NT + tile_of) * 2 + (1 - lo))
    order = np.lexsort((slot_in, key))
    gsizes = np.bincount(key, minlength=NCORES * NT * 2).reshape(NCORES, NT, 2)
    sr = src_row[order]
    sl = slot_in[order]
    goff = np.concatenate([[0], np.cumsum(gsizes.reshape(-1))])

    # per-(tile, half) chunk counts: uniform across cores, window-feasible
    cl_sec = np.zeros((NT, 2), np.int32)
    packs = {}  # (c, t, half) -> list of per-chunk slot-position lists
    for t in range(NT):
        for half in range(2):
            cl = max(1, int(np.ceil(gsizes[:, t, half].max() / 128)))
            while True:
                ok = True
                trial = {}
                for c in range(NCORES):
                    g = (c * NT + t) * 2 + half
                    ss = sl[goff[g]:goff[g + 1]]
                    res = _fill_section(ss, cl)
                    if res is None:
                        ok = False
                        break
                    trial[c] = res
                if ok:
                    break
                cl += 1
                assert cl <= 24, "window packing failed to converge"
            cl_sec[t, half] = cl
            for c in range(NCORES):
                packs[(c, t, half)] = trial[c]

    ch_t = (cl_sec[:, 0] + cl_sec[:, 1])
    pc_cols = int(ch_t.sum())
    # per-tile blob cols (bf16 units):
    #   [idx (CH*8, int16) | dstem (CH, bf16) | S (CH*64, fp8-packed)]
    blob_cols = pc_cols * 73
    blob = np.zeros((NCORES, 128, blob_cols), BF)

    for c in range(NCORES):
        bo = 0
        for t in range(NT):
            CH = int(ch_t[t])
            bidx = blob[c][:, bo:bo + CH * 8].view(np.int16)
            bpc = blob[c][:, bo + CH * 8:bo + CH * 9]
            bs = blob[c][:, bo + CH * 9:bo + CH * 73].view(ml_dtypes.float8_e4m3)
            ci_g = 0
            for half in range(2):
                g = (c * NT + t) * 2 + half
                base = goff[g]
                w0 = _w0_sched(cl_sec[t, half])
                srows = sr[base:goff[g + 1]]
                for ci, take in enumerate(packs[(c, t, half)]):
                    aidx = np.zeros(128, np.int16)
                    dslot = np.full(128, -1.0, np.float32)
                    for j, p in enumerate(take):
                        aidx[j] = srows[p] - (HALF if half else 0)
                        dslot[j] = sl[base + p]
                    bidx[:, ci_g * 8:(ci_g + 1) * 8] = _wrap_idx(aidx)
                    bpc[:, ci_g] = (dslot - w0[ci]).astype(BF)
                    sm = np.zeros((128, 128), np.float32)
                    valid = dslot >= 0
                    sm[dslot[valid].astype(np.int32), np.nonzero(valid)[0]] = 1.0
                    bs[:, ci_g * 128:(ci_g + 1) * 128] = sm.astype(ml_dtypes.float8_e4m3)
                    ci_g += 1
            bo += CH * 73

    return dict(cl_sec=cl_sec, ch_t=ch_t, pc_cols=pc_cols, blob=blob,
                perm_rows=perm_rows)


# ----------------------------------------------------------------------------
# device program
# ----------------------------------------------------------------------------

DEBUG = False


def _build_program(cl_key, single_core=False):
    cl_sec = np.array(cl_key, np.int32).reshape(NT, 2)
    ch_t = cl_sec[:, 0] + cl_sec[:, 1]
    CHMAX = int(ch_t.max())
    pc_cols = int(ch_t.sum())
    nc = bacc.Bacc("TRN2", target_bir_lowering=False, debug=False,
                   num_devices=1 if single_core else NCORES)
    f32, bf16, i16, f8 = DT.float32, DT.bfloat16, DT.int16, DT.float8e4
    dbg_outs = {}

    def dbg(name, shape, dtype):
        if DEBUG:
            dbg_outs[name] = nc.dram_tensor(name, shape, dtype, kind="ExternalOutput")
            return dbg_outs[name]
        return None

    xt = nc.dram_tensor("xt", [IN, SLOTS], f32, kind="ExternalInput")
    w1 = nc.dram_tensor("w1", [IN, IN], bf16, kind="ExternalInput")
    att1 = nc.dram_tensor("att1", [IN, 4], f32, kind="ExternalInput")
    w2sd = nc.dram_tensor("w2sd", [128, 2, OUT + 2], bf16, kind="ExternalInput")
    b1r = nc.dram_tensor("b1r", [128, IN], bf16, kind="ExternalInput")
    iotar = nc.dram_tensor("iotar", [128, W * CHMAX], bf16, kind="ExternalInput")
    ident = nc.dram_tensor("ident", [128, 128], bf16, kind="ExternalInput")
    blob_d = nc.dram_tensor("blob", [128, pc_cols * 73], bf16, kind="ExternalInput")
    out_d = nc.dram_tensor("out", [SLOTS, OUT], f32, kind="ExternalOutput")

    with tile.TileContext(nc) as tc:
        with (
            tc.tile_pool(name="const", bufs=1) as cst,
            tc.tile_pool(name="dram", bufs=1, space="DRAM") as dram,
        ):
            # long-lived small tiles
            w2t = cst.tile([128, 2, OUT + 2], bf16)
            nc.sync.dma_start(w2t[:], w2sd[:])
            b1t = cst.tile([128, IN], bf16)
            iot = cst.tile([128, W, CHMAX], bf16)
            idt = cst.tile([128, 128], bf16)
            zrow = cst.tile([1, 258], bf16)
            zcol = cst.tile([1, 128], bf16)
            nc.vector.memset(zrow[:], 0.0)
            nc.vector.memset(zcol[:], 0.0)
            nc.sync.dma_start(b1t[:], b1r[:])
            nc.sync.dma_start(iot[:].rearrange("p d c -> p (d c)"), iotar[:])
            nc.sync.dma_start(idt[:], ident[:])
            ad1 = cst.tile([128, 6 * NT], f8)   # layer1 a_d hi/mid/lo per tile
            ad2 = cst.tile([128, 3 * NT], f8)   # layer2 a_d hi/mid/lo per tile
            ad2f32 = cst.tile([128, NT], f32)
            adh32 = cst.tile([128, NT], f32)

            table1_self = dram.tile([SLOTS, EW1], bf16)
            table1_full = dram.tile([ROWS, EW1], bf16)
            table2_self = dram.tile([SLOTS, EW2], bf16)
            table2_full = dram.tile([ROWS, EW2], bf16)

            # ---------------- phase A (layer 1 projection) ----------------
            with (
                tc.tile_pool(name="pa", bufs=1) as pa,
                tc.tile_pool(name="pa2", bufs=4) as pa2,
                tc.tile_pool(name="pap", bufs=3, space="PSUM") as pap,
            ):
                xt32 = pa.tile([128, 2, SLOTS], f32)
                xtbf = pa.tile([128, 2, SLOTS], bf16)
                w1t = pa.tile([128, 2, IN], bf16)
                a1t = pa.tile([128, 2, 4], f32)
                ad32 = pa.tile([128, 2 * NT], f32)
                hi32 = pa.tile([128, 2 * NT], f32)
                nc.sync.dma_start(xt32[:], xt[:].rearrange("(a b) c -> b a c", b=128))
                nc.sync.dma_start(w1t[:], w1[:].rearrange("(a b) c -> b a c", b=128))
                nc.sync.dma_start(a1t[:], att1[:].rearrange("(a b) c -> b a c", b=128))
                nc.vector.tensor_copy(xtbf[:], xt32[:])

                for t in range(NT):
                    n0 = t * 128
                    psA = pap.tile([128, IN], f32, tag="psA")
                    psB = pap.tile([128, 4], f32, tag="psB")
                    for kc in range(2):
                        nc.tensor.matmul(psA[:], xtbf[:, kc, n0:n0 + 128],
                                         w1t[:, kc, :], start=(kc == 0), stop=(kc == 1))
                    for kc in range(2):
                        nc.tensor.matmul(psB[:], xt32[:, kc, n0:n0 + 128],
                                         a1t[:, kc, :], start=(kc == 0), stop=(kc == 1))
                    if t % 4 == 0:
                        t1 = pa2.tile([128, 4, EW1], bf16, tag="t1")
                    tm = t % 4
                    nc.scalar.copy(t1[:, tm, 1:129], psA[:, 0:128])
                    nc.scalar.copy(t1[:, tm, 130:258], psA[:, 128:256])
                    nc.vector.memset(t1[:, tm, 0:1], 1.0)
                    nc.vector.memset(t1[:, tm, 129:130], 1.0)
                    t1f = t1[:].bitcast(f32)
                    nc.vector.tensor_copy(t1f[:, tm, 130:132], psB[:, 0:2])
                    nc.vector.tensor_copy(ad32[:, 2 * t:2 * t + 2], psB[:, 2:4])
                    if t % 4 == 3 or t == NT - 1:
                        tb = t - tm
                        nc.sync.dma_start(
                            table1_self[tb * 128:(t + 1) * 128, :].rearrange(
                                "(a b) c -> b a c", b=128),
                            t1[:, 0:tm + 1, :])
                # batched a_d 3-way fp8 split into ad1 [hi0 hi1 mid0 mid1 lo0 lo1]
                ad1v = ad1[:].rearrange("p (t six) -> p t six", six=6)
                ad32v = ad32[:].rearrange("p (t two) -> p t two", two=2)
                hi32v = hi32[:].rearrange("p (t two) -> p t two", two=2)
                nc.vector.tensor_copy(ad1v[:, :, 0:2], ad32v)
                nc.vector.tensor_copy(hi32v, ad1v[:, :, 0:2])
                nc.vector.tensor_sub(ad32[:], ad32[:], hi32[:])
                nc.vector.tensor_copy(ad1v[:, :, 2:4], ad32v)
                nc.vector.tensor_copy(hi32v, ad1v[:, :, 2:4])
                nc.vector.tensor_sub(ad32[:], ad32[:], hi32[:])
                nc.vector.tensor_copy(ad1v[:, :, 4:6], ad32v)

            if DEBUG:
                d_t1 = dbg("d_t1self", [SLOTS, EW1], bf16)
                nc.sync.dma_start(d_t1[:], table1_self[:])
            if single_core:
                nc.sync.dma_start(table1_full[0:SLOTS, :], table1_self[:])
            else:
                nc.gpsimd.collective_compute(
                    "AllGather", mybir.AluOpType.bypass,
                    replica_groups=[list(range(NCORES))],
                    ins=[table1_self.opt()], outs=[table1_full.opt()],
                )

            # ---------------- layer 1 aggregation + layer 2 projection ----
            with (
                tc.tile_pool(name="ag", bufs=4) as ag,
                tc.tile_pool(name="agp", bufs=2, space="PSUM") as agp,
                tc.tile_pool(name="agp1", bufs=1, space="PSUM") as agp1,
            ):
                io = 0
                po = 0
                for t in range(NT):
                    n0 = t * 128
                    cl_lo, cl_hi = int(cl_sec[t, 0]), int(cl_sec[t, 1])
                    CH = cl_lo + cl_hi
                    C = CH * 128
                    w0s = _w0_sched(cl_lo) + _w0_sched(cl_hi)
                    nlo, nhi = cl_lo * 128, cl_hi * 128
                    blob = ag.tile([128, CH * 73], bf16, tag="blob")
                    nc.sync.dma_start(blob[:], blob_d[:, io:io + CH * 73])
                    idx = blob[:, 0:CH * 8].bitcast(i16)
                    dpc = blob[:, CH * 8:CH * 9]
                    S = blob[:, CH * 9:CH * 73].bitcast(f8)
                    G = ag.tile([128, CH, EW1], bf16, tag="G")
                    nc.gpsimd.dma_gather(G[:, 0:cl_lo, :], table1_full[0:HALF, :],
                                         idx[:, 0:cl_lo * 8], nlo, nlo, EW1)
                    nc.gpsimd.dma_gather(G[:, cl_lo:CH, :], table1_full[HALF:ROWS, :],
                                         idx[:, cl_lo * 8:CH * 8], nhi, nhi, EW1)
                    # T[e, dw, c] = (dst_slot_rel[e, c] == dw), all-bf16 packed
                    T = ag.tile([128, W, CHMAX], bf16, tag="T")
                    dpc_b = bass.AP(dpc.tensor, dpc.offset,
                                    [dpc.ap[0], [0, W], dpc.ap[1]])
                    nc.vector.tensor_tensor(T[:, :, 0:CH], dpc_b, iot[:, :, 0:CH],
                                            mybir.AluOpType.is_equal)
                    # a_d expansion: psE[:, 4c:4c+4] = S_c[w0:w0+W].T @ ad1[w0:w0+W]
                    psE = agp.tile([128, 6 * CHMAX], f32, tag="psE")
                    for cc in range(CH):
                        w0 = w0s[cc]
                        nc.tensor.matmul(psE[:, 6 * cc:6 * cc + 6],
                                         S[w0:w0 + W, cc * 128:(cc + 1) * 128],
                                         ad1[w0:w0 + W, 6 * t:6 * t + 6],
                                         start=True, stop=True)
                    # alpha = leaky(a_s + a_d), ex = exp(alpha) (bf16, h-major)
                    aw = ag.tile([128, 2, CH], f32, tag="aw")
                    Gf = G[:].bitcast(f32)
                    for h in range(2):
                        pse_h = bass.AP(psE.tensor, psE[:].offset + h,
                                        [psE[:].ap[0], [6, CH], [2, 3]])
                        nc.vector.tensor_reduce(aw[:, h, :], pse_h,
                                                mybir.AxisListType.X,
                                                mybir.AluOpType.add)
                        nc.vector.tensor_add(aw[:, h, :], aw[:, h, :],
                                             Gf[:, :, 130 + h])
                    al = ag.tile([128, 2, CH], f32, tag="al")
                    nc.vector.tensor_scalar_mul(al[:], aw[:], NEG)
                    nc.vector.tensor_max(al[:], al[:], aw[:])
                    ex = ag.tile([128, 2, CH], bf16, tag="ex")
                    nc.scalar.activation(ex[:], al[:], mybir.ActivationFunctionType.Exp)
                    # Sw[h][e, dw, c] = T[e, dw, c] * ex[e, h, c]  (all-bf16 packed)
                    Sw = ag.tile([128, 2, W, CHMAX], bf16, tag="Sw")
                    T_b = bass.AP(T.tensor, T[:].offset,
                                  [T[:].ap[0], [0, 2], [CHMAX, W], [1, CH]])
                    ex_b = bass.AP(ex.tensor, ex[:].offset,
                                   [ex[:].ap[0], [CH, 2], [0, W], [1, CH]])
                    nc.vector.tensor_mul(Sw[:, :, :, 0:CH], T_b, ex_b)
                    # aggregation into dst windows; psum cleared by a zero-matmul
                    psO = agp.tile([128, 2, 129], f32, tag="psO")
                    nc.tensor.matmul(psO[:].rearrange("p h n -> p (h n)"),
                                     zcol[:], zrow[:, 0:258],
                                     start=True, stop=False, skip_group_check=True)
                    for h in range(2):
                        for cc in range(CH):
                            w0 = w0s[cc]
                            nc.tensor.matmul(psO[w0:w0 + W, h, :], Sw[:, h, :, cc],
                                             G[:, cc, 129 * h:129 * (h + 1)],
                                             start=False,
                                             stop=(h == 1 and cc == CH - 1),
                                             skip_group_check=True)
                    if DEBUG and t == 0:
                        d_g = dbg("d_G", [128, CH, EW1], bf16)
                        nc.sync.dma_start(d_g[:], G[:])
                        d_t = dbg("d_T", [128, 128, CH], bf16)
                        nc.sync.dma_start(d_t[:], T[:])
                        d_s = dbg("d_S", [128, C], bf16)
                        nc.sync.dma_start(d_s[:], S[:])
                        d_aw = dbg("d_aw", [128, 2, CH], f32)
                        nc.sync.dma_start(d_aw[:], aw[:])
                        d_ex = dbg("d_ex", [128, 2, CH], bf16)
                        nc.sync.dma_start(d_ex[:], ex[:])
                        pse_sb = ag.tile([128, 4 * CH], f32, tag="pse_sb")
                        nc.vector.tensor_copy(pse_sb[:], psE[:])
                        d_pse = dbg("d_psE", [128, 4 * CH], f32)
                        nc.sync.dma_start(d_pse[:], pse_sb[:])
                        pso_sb = ag.tile([128, 2, 129], f32, tag="pso_sb")
                        nc.vector.tensor_copy(pso_sb[:], psO[:])
                        d_pso = dbg("d_psO", [128, 2, 129], f32)
                        nc.sync.dma_start(d_pso[:], pso_sb[:])
                    rec = ag.tile([128, 2], f32, tag="rec")
                    nc.vector.tensor_scalar_add(rec[:], psO[:, :, 0], 1e-16)
                    nc.vector.reciprocal(rec[:], rec[:])
                    # hag = psO * rec (ACT per-partition scale), bf16 out
                    hag = ag.tile([128, IN], bf16, tag="hag")
                    hag3 = hag[:].rearrange("p (h d) -> p h d", h=2)
                    for h in range(2):
                        nc.scalar.activation(hag3[:, h, :], psO[:, h, 1:129],
                                             mybir.ActivationFunctionType.Copy,
                                             scale=rec[:, h:h + 1])
                    nc.vector.tensor_add(hag[:], hag[:], b1t[:])
                    # elu = max(exp(min(h,0))-1, h), bf16
                    e1 = ag.tile([128, IN], bf16, tag="e1")
                    nc.vector.tensor_scalar_min(e1[:], hag[:], 0.0)
                    nc.scalar.activation(e1[:], e1[:], mybir.ActivationFunctionType.Exp)
                    nc.vector.tensor_scalar_sub(e1[:], e1[:], 1.0)
                    nc.vector.tensor_max(e1[:], e1[:], hag[:])   # e1 = h_elu bf16
                    # transpose h_elu, project with [W2 | v2s | v2d]
                    psT = agp.tile([128, 2, 128], bf16, tag="psT")
                    for kc in range(2):
                        nc.tensor.transpose(psT[:, kc, :], e1[:, kc * 128:(kc + 1) * 128],
                                            idt[:])
                    ebT = ag.tile([128, 2, 128], bf16, tag="ebT")
                    nc.scalar.copy(ebT[:], psT[:])
                    ps2 = agp.tile([128, OUT + 2], f32, tag="ps2")
                    for kc in range(2):
                        nc.tensor.matmul(ps2[:], ebT[:, kc, :], w2t[:, kc, :],
                                         start=(kc == 0), stop=(kc == 1))
                    nc.vector.tensor_copy(ad2f32[:, t:t + 1], ps2[:, 129:130])
                    t2 = ag.tile([128, EW2], bf16, tag="t2")
                    nc.scalar.copy(t2[:, 1:129], ps2[:, 0:128])
                    nc.vector.memset(t2[:, 0:1], 1.0)
                    t2f = t2[:].bitcast(f32)
                    nc.vector.tensor_copy(t2f[:, 65:66], ps2[:, 128:129])
                    nc.sync.dma_start(table2_self[n0:n0 + 128, :], t2[:])
                    io += CH * 73
                    if DEBUG and t == 0:
                        d_hag = dbg("d_hag", [128, IN], bf16)
                        nc.sync.dma_start(d_hag[:], hag[:])
                        d_helu = dbg("d_helu", [128, IN], bf16)
                        nc.sync.dma_start(d_helu[:], e1[:])
                        d_t2 = dbg("d_t2", [128, EW2], bf16)
                        nc.sync.dma_start(d_t2[:], t2[:])
                        

            # batched a_d2 3-way fp8 split
            ad2v = ad2[:].rearrange("p (t three) -> p t three", three=3)
            nc.vector.tensor_copy(ad2v[:, :, 0], ad2f32[:])
            nc.vector.tensor_copy(adh32[:], ad2v[:, :, 0])
            nc.vector.tensor_sub(ad2f32[:], ad2f32[:], adh32[:])
            nc.vector.tensor_copy(ad2v[:, :, 1], ad2f32[:])
            nc.vector.tensor_copy(adh32[:], ad2v[:, :, 1])
            nc.vector.tensor_sub(ad2f32[:], ad2f32[:], adh32[:])
            nc.vector.tensor_copy(ad2v[:, :, 2], ad2f32[:])
            if single_core:
                nc.sync.dma_start(table2_full[0:SLOTS, :], table2_self[:])
            else:
                nc.gpsimd.collective_compute(
                    "AllGather", mybir.AluOpType.bypass,
                    replica_groups=[list(range(NCORES))],
                    ins=[table2_self.opt()], outs=[table2_full.opt()],
                )

            # ---------------- layer 2 aggregation ----------------
            with (
                tc.tile_pool(name="bg", bufs=4) as bg,
                tc.tile_pool(name="bgp", bufs=2, space="PSUM") as bgp,
            ):
                io = 0
                po = 0
                for t in range(NT):
                    n0 = t * 128
                    cl_lo, cl_hi = int(cl_sec[t, 0]), int(cl_sec[t, 1])
                    CH = cl_lo + cl_hi
                    C = CH * 128
                    w0s = _w0_sched(cl_lo) + _w0_sched(cl_hi)
                    nlo, nhi = cl_lo * 128, cl_hi * 128
                    blob = bg.tile([128, CH * 73], bf16, tag="blob")
                    nc.sync.dma_start(blob[:], blob_d[:, io:io + CH * 73])
                    idx = blob[:, 0:CH * 8].bitcast(i16)
                    dpc = blob[:, CH * 8:CH * 9]
                    S = blob[:, CH * 9:CH * 73].bitcast(f8)
                    G = bg.tile([128, CH, EW2], bf16, tag="G2")
                    nc.gpsimd.dma_gather(G[:, 0:cl_lo, :], table2_full[0:HALF, :],
                                         idx[:, 0:cl_lo * 8], nlo, nlo, EW2)
                    nc.gpsimd.dma_gather(G[:, cl_lo:CH, :], table2_full[HALF:ROWS, :],
                                         idx[:, cl_lo * 8:CH * 8], nhi, nhi, EW2)
                    T = bg.tile([128, W, CHMAX], bf16, tag="T")
                    dpc_b = bass.AP(dpc.tensor, dpc.offset,
                                    [dpc.ap[0], [0, W], dpc.ap[1]])
                    nc.vector.tensor_tensor(T[:, :, 0:CH], dpc_b, iot[:, :, 0:CH],
                                            mybir.AluOpType.is_equal)
                    psE = bgp.tile([128, 3 * CHMAX], f32, tag="psE2")
                    for cc in range(CH):
                        w0 = w0s[cc]
                        nc.tensor.matmul(psE[:, 3 * cc:3 * cc + 3],
                                         S[w0:w0 + W, cc * 128:(cc + 1) * 128],
                                         ad2[w0:w0 + W, 3 * t:3 * t + 3],
                                         start=True, stop=True)
                    aw = bg.tile([128, CH], f32, tag="aw2")
                    Gf = G[:].bitcast(f32)
                    pse_3 = bass.AP(psE.tensor, psE[:].offset,
                                    [psE[:].ap[0], [3, CH], [1, 3]])
                    nc.vector.tensor_reduce(aw[:], pse_3, mybir.AxisListType.X,
                                            mybir.AluOpType.add)
                    nc.vector.tensor_add(aw[:], aw[:], Gf[:, :, 65])
                    al = bg.tile([128, CH], f32, tag="al2")
                    nc.vector.tensor_scalar_mul(al[:], aw[:], NEG)
                    nc.vector.tensor_max(al[:], al[:], aw[:])
                    ex = bg.tile([128, CH], bf16, tag="ex2")
                    nc.scalar.activation(ex[:], al[:], mybir.ActivationFunctionType.Exp)
                    Sw = bg.tile([128, W, CHMAX], bf16, tag="Sw")
                    ex_b = bass.AP(ex.tensor, ex[:].offset,
                                   [ex[:].ap[0], [0, W], [1, CH]])
                    nc.vector.tensor_mul(Sw[:, :, 0:CH], T[:, :, 0:CH], ex_b)
                    psO = bgp.tile([128, 129], f32, tag="psO2")
                    nc.tensor.matmul(psO[:], zcol[:], zrow[:, 0:129],
                                     start=True, stop=False, skip_group_check=True)
                    for cc in range(CH):
                        w0 = w0s[cc]
                        nc.tensor.matmul(psO[w0:w0 + W, :], Sw[:, :, cc],
                                         G[:, cc, 0:129],
                                         start=False, stop=(cc == CH - 1),
                                         skip_group_check=True)
                    if DEBUG and t == 0:
                        pso2_sb = bg.tile([128, 129], f32, tag="pso2_sb")
                        nc.vector.tensor_copy(pso2_sb[:], psO[:])
                        d_pso2 = dbg("d_psO2", [128, 129], f32)
                        nc.sync.dma_start(d_pso2[:], pso2_sb[:])
                        d_ex2 = dbg("d_ex2", [128, CH], bf16)
                        nc.sync.dma_start(d_ex2[:], ex[:])
                        d_aw2 = dbg("d_aw2", [128, CH], f32)
                        nc.sync.dma_start(d_aw2[:], aw[:])
                    rec = bg.tile([128, 1], f32, tag="rec2")
                    nc.vector.tensor_scalar_add(rec[:], psO[:, 0:1], 1e-16)
                    nc.vector.reciprocal(rec[:], rec[:])
                    oo = bg.tile([128, OUT], f32, tag="oo")
                    nc.scalar.activation(oo[:], psO[:, 1:129],
                                         mybir.ActivationFunctionType.Copy,
                                         scale=rec[:, 0:1])
                    nc.sync.dma_start(out_d[n0:n0 + 128, :], oo[:])
                    io += CH * 73

    nc.compile()
    return nc


# ----------------------------------------------------------------------------
# entry point
# ----------------------------------------------------------------------------

_CACHE = {}


def kernel(x, edge_index, W1, att_src1, att_dst1, b1, W2, att_src2, att_dst2, b2,
           _want_trace=False):
    x = np.asarray(x, np.float32)
    edge_index = np.asarray(edge_index)
    W1 = np.asarray(W1, np.float32)
    W2 = np.asarray(W2, np.float32)

    pp = _preprocess(edge_index)
    cl_key = tuple(int(v) for v in pp["cl_sec"].reshape(-1))
    CHMAX = int(pp["ch_t"].max())

    # folded attention vectors (layer1: per head v = W1[:, h] @ att)
    att1 = np.zeros((IN, 4), np.float64)
    for h in range(H1):
        att1[:, h] = W1[:, h * HID:(h + 1) * HID].astype(np.float64) @ np.asarray(att_src1, np.float64)[h]
        att1[:, 2 + h] = W1[:, h * HID:(h + 1) * HID].astype(np.float64) @ np.asarray(att_dst1, np.float64)[h]
    att1 = att1.astype(np.float32)
    v2s = (W2.astype(np.float64) @ np.asarray(att_src2, np.float64)[0]).astype(np.float32)
    v2d = (W2.astype(np.float64) @ np.asarray(att_dst2, np.float64)[0]).astype(np.float32)
    w2sd = np.zeros((128, 2, OUT + 2), np.float32)
    for kc in range(2):
        w2sd[:, kc, 0:OUT] = W2[kc * 128:(kc + 1) * 128, :]
        w2sd[:, kc, OUT] = v2s[kc * 128:(kc + 1) * 128]
        w2sd[:, kc, OUT + 1] = v2d[kc * 128:(kc + 1) * 128]
    w2sd = w2sd.astype(BF)

    if cl_key not in _CACHE:
        _CACHE[cl_key] = _build_program(cl_key)
    nc = _CACHE[cl_key]

    # per-core inputs
    inv_half = np.zeros(SLOTS * NCORES, np.int64)
    perm = pp["perm_rows"]
    xt_all = np.zeros((NCORES, IN, SLOTS), np.float32)
    for c in range(NCORES):
        nodes = np.arange(c * NS, (c + 1) * NS)
        cols = perm[nodes] - c * SLOTS
        xt_all[c][:, cols] = x[nodes].T

    iotar = np.tile(np.arange(W, dtype=np.float32)[None, :, None],
                    (128, 1, CHMAX)).astype(BF).reshape(128, W * CHMAX)
    ident = np.eye(128, dtype=BF)
    b1rep = np.tile(np.asarray(b1, np.float32)[None, :], (128, 1)).astype(BF)
    w1bf = W1.astype(BF)

    in_maps = []
    for c in range(NCORES):
        in_maps.append({
            "xt": xt_all[c], "w1": w1bf, "att1": att1, "w2sd": w2sd,
            "b1r": b1rep,
            "iotar": iotar, "ident": ident,
            "blob": pp["blob"][c],
        })

    res = bass_utils.run_bass_kernel_spmd(
        nc, in_maps, core_ids=list(range(NCORES)), trace=_want_trace)

    out = np.zeros((N, OUT), np.float32)
    for c in range(NCORES):
        o = res.results[c]["out"]
        nodes = np.arange(c * NS, (c + 1) * NS)
        out[nodes] = o[perm[nodes] - c * SLOTS]
    out += np.asarray(b2, np.float32)[None, :]

    kernel._last_exec_ns = res.exec_time_ns
    kernel._last_trace = res.instructions_and_trace
    kernel._last_results = res.results
    return out

